# revision 1
# baseline (speedup 1.0000x reference)
"""Trainium2 Bass kernel for nn_DiagnosticRNN (LSTM B=2048,T=128,V=25,H=512
-> FC 100), 8-way batch-data-parallel across NeuronCores.

Strategy (v2)
-------------
Data-parallel over batch: each of the 8 cores runs the full T=128 LSTM
recurrence on BS=256 rows with replicated weights, fully fused on-chip.

Per-core per-timestep (bf16 operands, fp32 PSUM):
  gates[4H, BS] = W_hh_perm @ h_{t-1} + W_ihaug_perm @ [x_t; 1]
  PSUM layout groups gates by TYPE so ScalarE runs few, wide activations
  (ACT has a ~300ns fixed cost per instruction):
    pif[P] (2 banks, 1024 f32): (i_{2P} | i_{2P+1} | f_{2P} | f_{2P+1})
    pgo[P] (2 banks, 1024 f32): (g_{2P} | g_{2P+1} | o_{2P} | o_{2P+1})
  -> 8 ACT instructions/step: 2x sigmoid(if, 1024), 2x tanh(g, 512),
     2x sigmoid(o, 512), 2x tanh(c, 512).
  j-pair pipelining: the P=0 chain (ACT/DVE/GpSimd) runs while the PE
  fills P=1's banks; h is double-buffered by step parity so next step's
  matmuls never WAR-block the h writes.
  x-term: thin K=26 matmuls, 2-way row-group concurrency (x replicated
  at partition offsets 0/32 by on-chip DMA from a compact [26, .] HBM
  tensor). 4-way row groups measured SLOWER on HW (+2.6us/step) and
  odd-cycle groupings crashed the device, so XNR=2.
  hh MM order: pif groups k-inner (so k=3, which needs the other pair's
  h chunks from the previous step, issues as late as possible); pgo
  groups g-before-o so tanh(g) - which gates the c-chain - starts early.

FC epilogue: out[100, BS] = W_fc @ h_T (+b_fc via ACT Identity bias);
host transposes to [BS, 100].
"""

import numpy as np
import ml_dtypes

import concourse.bacc as bacc
import concourse.mybir as mybir
import concourse.tile as tile
from concourse.bass_utils import run_bass_kernel_spmd

F32 = mybir.dt.float32
BF16 = mybir.dt.bfloat16
AF = mybir.ActivationFunctionType

B, T, V = 2048, 128, 25
H = 512
NCLS = 100
CORES = 8
BS = B // CORES          # 256 batch rows per core
KT = H // 128            # 4 k-tiles (h chunks)
KV = V + 1               # 26 contraction rows for the x-term (ones row)
# x-MM row-group replication count. 2 measured best on HW: consecutive
# x-MMs in different row-groups overlap; 4 groups triggered a large HW
# slowdown (and row-group counts that don't divide the quad pattern
# crashed the device), 1 serializes. x is replicated at partition
# offsets 0/32 only.
XNR = 2

# m-tile enumeration. q: 0=i 1=f 2=g 3=o (PyTorch gate order).
# x-tile issue order (also the wih column layout). Consecutive x-MMs run
# CONCURRENTLY on the PE when their row-groups differ, so each quad must
# target 4 distinct PSUM banks (concurrent writes to one bank are a fatal
# PSUM collision): order (i,f,g,o) x dj -> banks (ifA, ifB, goA, goB).
_XTILES = []
for _P in (0, 1):
    for _dj in (0, 1):
        for _q in (0, 1, 2, 3):
            _XTILES.append((_q, 2 * _P + _dj))
# hh-tile issue order (also the whh column layout): per P: IF group
# k-inner; GO group q-outer (all g, then all o), k-inner per q.
_HTILES = []
for _P in (0, 1):
    for _k in range(KT):                   # IF group, k-inner
        for _q in (0, 1):
            for _dj in (0, 1):
                _HTILES.append((_q, 2 * _P + _dj, _k))
    for _q in (2, 3):                      # GO group, g first then o
        for _k in range(KT):
            for _dj in (0, 1):
                _HTILES.append((_q, 2 * _P + _dj, _k))


def _psum_slot(q, dj):
    """(tile_kind, col) for m-tile (q, j=2P+dj): which 256-col slot of
    pif[P]/pgo[P]. Layout: pif=(i0 i1 f0 f1), pgo=(g0 g1 o0 o1)."""
    if q in (0, 1):
        return "if", (q * 2 + dj) * BS
    return "go", ((q - 2) * 2 + dj) * BS


def pack_host(messages, W_ih, W_hh, b_ih, b_hh, W_fc, b_fc):
    whh = np.zeros((128, len(_HTILES) * 128), np.float32)
    for idx, (q, j, k) in enumerate(_HTILES):
        rows = slice(q * H + j * 128, q * H + (j + 1) * 128)
        whh[:, idx * 128:(idx + 1) * 128] = W_hh[rows, k * 128:(k + 1) * 128].T
    whh = whh.astype(ml_dtypes.bfloat16)

    bias = b_ih + b_hh
    wih = np.zeros((128, len(_XTILES) * 128), np.float32)
    for xm, (q, j) in enumerate(_XTILES):
        rows = slice(q * H + j * 128, q * H + (j + 1) * 128)
        r = xm % XNR
        wih[r * 32:r * 32 + V, xm * 128:(xm + 1) * 128] = W_ih[rows].T
        wih[r * 32 + V, xm * 128:(xm + 1) * 128] = bias[rows]
    wih = wih.astype(ml_dtypes.bfloat16)

    wfc = np.zeros((128, KT * NCLS), np.float32)
    for k in range(KT):
        wfc[:, k * NCLS:(k + 1) * NCLS] = W_fc.T[k * 128:(k + 1) * 128]
    wfc = wfc.astype(ml_dtypes.bfloat16)
    bfc = b_fc.astype(np.float32).reshape(NCLS, 1)

    in_maps = []
    for c in range(CORES):
        shard = messages[c * BS:(c + 1) * BS]                # [BS, T, V]
        xc = np.empty((KV, T, BS), np.float32)
        xc[:V] = shard.transpose(2, 1, 0)
        xc[V] = 1.0
        xc = xc.reshape(KV, T * BS).astype(ml_dtypes.bfloat16)
        in_maps.append({"x_c": xc, "whh": whh, "wih": wih,
                        "wfc": wfc, "bfc": bfc})
    return in_maps


def build(reps=1, nsteps=T):
    nc = bacc.Bacc("TRN2", target_bir_lowering=False, debug=False)

    x_dram = nc.dram_tensor("x_c", [KV, T * BS], BF16,
                            kind="ExternalInput").ap()
    whh_dram = nc.dram_tensor("whh", [128, len(_HTILES) * 128], BF16,
                              kind="ExternalInput").ap()
    wih_dram = nc.dram_tensor("wih", [128, len(_XTILES) * 128], BF16,
                              kind="ExternalInput").ap()
    wfc_dram = nc.dram_tensor("wfc", [128, KT * NCLS], BF16,
                              kind="ExternalInput").ap()
    bfc_dram = nc.dram_tensor("bfc", [NCLS, 1], F32,
                              kind="ExternalInput").ap()
    out_dram = nc.dram_tensor("out", [NCLS, BS], F32,
                              kind="ExternalOutput").ap()

    with tile.TileContext(nc) as tc:
        with (
            tc.tile_pool(name="const", bufs=1) as cpool,
            tc.tile_pool(name="xbuf", bufs=1) as xpool,
            tc.tile_pool(name="state", bufs=1) as spool,
            tc.tile_pool(name="psum", bufs=1, space="PSUM") as ppool,
            tc.tile_pool(name="work", bufs=1) as wpool,
        ):
            whh_sb = cpool.tile([128, len(_HTILES) * 128], BF16)
            wih_sb = cpool.tile([128, len(_XTILES) * 128], BF16)
            wfc_sb = cpool.tile([128, KT * NCLS], BF16)
            bfc_sb = cpool.tile([NCLS, 1], F32)
            x4_sb = xpool.tile([128, T * BS], BF16)

            h_sb = [spool.tile([128, KT * BS], BF16, name=f"h{i}")
                    for i in range(2)]
            c_sb = spool.tile([128, KT * BS], BF16)

            # weights: a few parallel DMAs
            wc = len(_HTILES) * 128 // 4
            for i in range(4):
                nc.sync.dma_start(whh_sb[:, i * wc:(i + 1) * wc],
                                  whh_dram[:, i * wc:(i + 1) * wc])
            nc.sync.dma_start(wih_sb[:], wih_dram[:])
            nc.sync.dma_start(wfc_sb[:], wfc_dram[:])
            nc.sync.dma_start(bfc_sb[:], bfc_dram[:])
            # x: stream in 8 T-chunks, replicated to XNR partition offsets
            xc_cols = T * BS // 8
            for i in range(8):
                cols = slice(i * xc_cols, (i + 1) * xc_cols)
                for r in range(XNR):
                    nc.sync.dma_start(x4_sb[r * 32:r * 32 + KV, cols],
                                      x_dram[:, cols])

            pif = [ppool.tile([128, 4 * BS], F32, name=f"pif{p}")
                   for p in range(2)]
            pgo = [ppool.tile([128, 4 * BS], F32, name=f"pgo{p}")
                   for p in range(2)]

            sif = [wpool.tile([128, 4 * BS], BF16, name=f"sif{p}")
                   for p in range(2)]
            g_t = [wpool.tile([128, 2 * BS], BF16, name=f"g{p}")
                   for p in range(2)]
            o_t = [wpool.tile([128, 2 * BS], BF16, name=f"o{p}")
                   for p in range(2)]
            ig_t = [wpool.tile([128, 2 * BS], BF16, name=f"ig{p}")
                    for p in range(2)]
            fc_t = [wpool.tile([128, 2 * BS], BF16, name=f"fc{p}")
                    for p in range(2)]
            tc_t = [wpool.tile([128, 2 * BS], BF16, name=f"tc{p}")
                    for p in range(2)]

            def x_mm(t, xm_base):
                """Issue the 4 x-term matmuls for one psum group."""
                xs = slice(t * BS, (t + 1) * BS)
                for n in range(4):
                    xm = xm_base + n
                    q, j = _XTILES[xm]
                    kind, col = _psum_slot(q, j % 2)
                    dst = (pif if kind == "if" else pgo)[j // 2]
                    r = xm % XNR
                    nc.tensor.matmul(
                        dst[:, col:col + BS],
                        wih_sb[r * 32:r * 32 + KV,
                               xm * 128:(xm + 1) * 128],
                        x4_sb[r * 32:r * 32 + KV, xs],
                        start=(col % 512 == 0), stop=False,
                        tile_position=(r * 32, 0),
                    )

            def hh_mm(hr, idx_base):
                """Issue 16 hh matmuls (one psum group) reading h buf hr."""
                for n in range(16):
                    q, j, k = _HTILES[idx_base + n]
                    kind, col = _psum_slot(q, j % 2)
                    dst = (pif if kind == "if" else pgo)[j // 2]
                    idx = idx_base + n
                    # stop on the last write to each bank
                    last = (k == KT - 1) and (col % 512 == BS)
                    nc.tensor.matmul(
                        dst[:, col:col + BS],
                        whh_sb[:, idx * 128:(idx + 1) * 128],
                        hr[:, k * BS:(k + 1) * BS],
                        start=False, stop=last,
                    )

            def chain(P, hw):
                """ACT/DVE/GpSimd chain for j-pair P, writing h buf hw."""
                s2 = slice(P * 2 * BS, (P + 1) * 2 * BS)
                nc.scalar.activation(sif[P][:], pif[P][:], AF.Sigmoid)
                nc.scalar.activation(g_t[P][:], pgo[P][:, 0:2 * BS],
                                     AF.Tanh)
                nc.vector.tensor_mul(ig_t[P][:], sif[P][:, 0:2 * BS],
                                     g_t[P][:])
                nc.gpsimd.tensor_mul(fc_t[P][:], sif[P][:, 2 * BS:4 * BS],
                                     c_sb[:, s2])
                nc.scalar.activation(o_t[P][:], pgo[P][:, 2 * BS:4 * BS],
                                     AF.Sigmoid)
                nc.vector.tensor_add(c_sb[:, s2], ig_t[P][:], fc_t[P][:])
                nc.scalar.activation(tc_t[P][:], c_sb[:, s2], AF.Tanh)
                nc.vector.tensor_mul(hw[:, s2], o_t[P][:], tc_t[P][:])

            for rep in range(reps):
                nc.vector.memset(h_sb[0][:], 0.0)
                nc.vector.memset(c_sb[:], 0.0)
                for t in range(nsteps):
                    hr, hw = h_sb[t % 2], h_sb[1 - t % 2]
                    x_mm(t, 0)       # x -> pif0+pgo0, dj=0
                    x_mm(t, 4)       # x -> pif0+pgo0, dj=1
                    hh_mm(hr, 0)     # hh -> pif0
                    hh_mm(hr, 16)    # hh -> pgo0
                    x_mm(t, 8)       # x -> pif1+pgo1, dj=0
                    x_mm(t, 12)      # x -> pif1+pgo1, dj=1
                    hh_mm(hr, 32)    # hh -> pif1
                    hh_mm(hr, 48)    # hh -> pgo1
                    chain(0, hw)
                    chain(1, hw)

            h_fin = h_sb[nsteps % 2]
            for k in range(KT):
                nc.tensor.matmul(
                    pif[0][0:NCLS, 0:BS],
                    wfc_sb[:, k * NCLS:(k + 1) * NCLS],
                    h_fin[:, k * BS:(k + 1) * BS],
                    start=(k == 0), stop=(k == KT - 1),
                )
            out_sb = cpool.tile([NCLS, BS], F32)
            nc.scalar.activation(out_sb[:], pif[0][0:NCLS, 0:BS],
                                 AF.Identity, bias=bfc_sb[:])
            nc.sync.dma_start(out_dram[:], out_sb[:])

    nc.compile()
    return nc


_NC_CACHE = None


def kernel(messages, W_ih, W_hh, b_ih, b_hh, W_fc, b_fc):
    """Full-input entry point: shard, run on 8 NeuronCores, gather."""
    global _NC_CACHE
    messages = np.asarray(messages, np.float32)
    W_ih = np.asarray(W_ih, np.float32)
    W_hh = np.asarray(W_hh, np.float32)
    b_ih = np.asarray(b_ih, np.float32)
    b_hh = np.asarray(b_hh, np.float32)
    W_fc = np.asarray(W_fc, np.float32)
    b_fc = np.asarray(b_fc, np.float32)

    in_maps = pack_host(messages, W_ih, W_hh, b_ih, b_hh, W_fc, b_fc)
    if _NC_CACHE is None:
        _NC_CACHE = build(1)
    res = run_bass_kernel_spmd(_NC_CACHE, in_maps, list(range(CORES)))
    outs = [np.ascontiguousarray(np.asarray(res.results[c]["out"]).T)
            for c in range(CORES)]
    return np.concatenate(outs, axis=0).astype(np.float32)



# revision 4
# speedup vs baseline: 1.3879x; 1.3879x over previous
"""Trainium2 Bass kernel for nn_DiagnosticRNN (LSTM B=2048,T=128,V=25,
H=512 -> FC 100), 8-way batch-data-parallel across NeuronCores.

Each core runs the full T=128 recurrence on BS=256 rows with replicated
weights, fully fused on-chip. Three optimizations over the 1175us bf16
baseline (same-session A/B-measured):

1. fp8e4 DoubleRow hh matmuls (-20%): gates[4H,BS] += W_hh @ h runs as
   32 DR MMs/step (2 fp8 weights per PE cell, K=256) instead of 64 bf16
   MMs; h is stored fp8e4 (a bf16 copy is written only on the last step
   for the FC epilogue). The x-term (K=26, N-bound not K-bound) stays
   bf16. HW rel err 0.0147 vs gate 0.02 (numpy-emulated first: 0.0148).
2. Weight scaling: W_hh, W_ih, bias are scaled by 32 on the host so fp8
   W_hh sits in e4m3's normal range (unscaled, ~28% of the N(0,1/512)
   weights are subnormal with ~10% quant error); descale is free via
   the ACT scale operand: func(x/32).
3. Dependency-cycle tuning (~3%): a const-h ablation showed pure PE
   streaming is only ~4.0us/step vs ~7 measured -- the recurrence
   dependency cycle (chainP(t) -> MMs reading hP -> psum -> chainP(t+1))
   binds, not LDWEIGHTS. So: f*c on DVE (issued before i*g; ~0.3us vs
   1.1 gpsimd) shortens chain latency, and DR tiles issue kp-outer (all
   kp0 = chain0 consumers first, all kp1 last) so each chain's latency
   overlaps the other half's matmul stream. Remaining structure is
   pinned jointly by ACT throughput (~5.4us/step: 5120 activation cols
   + 8x172c fixed) and the ~3.1us chain latency; batch-splitting the
   chains or merging ACT instrs trades one for the other and measured
   no better.

Tried and rejected (HW-measured): hybrid fp8/bf16 contraction (bf16 FWL
inactive in this path: 11% slower), DoubleRowSwInterleave (device
crash), schedule-only restructure of the bf16 baseline (throttled-PE
bound, 10% slower), 4-way x row-groups (prior session: slower/crashy).
"""

WS = 32.0                # weight scale (descaled in ACT)

import numpy as np
import ml_dtypes

import concourse.bacc as bacc
import concourse.mybir as mybir
import concourse.tile as tile
from concourse.bass_utils import run_bass_kernel_spmd

F32 = mybir.dt.float32
BF16 = mybir.dt.bfloat16
FP8 = mybir.dt.float8e4
AF = mybir.ActivationFunctionType
DR = mybir.MatmulPerfMode.DoubleRow

B, T, V = 2048, 128, 25
H = 512
NCLS = 100
CORES = 8
BS = B // CORES          # 256 batch rows per core
KT = H // 128            # 4 k-tiles (h chunks)
KP = 2                   # 2 k-pairs (256-wide DR contraction chunks)
KV = V + 1               # 26 contraction rows for the x-term (ones row)
XNR = 2                  # x row-group replication (see baseline docstring)

# x-tile issue order (also the wih column layout); same as baseline.
_XTILES = []
for _P in (0, 1):
    for _dj in (0, 1):
        for _q in (0, 1, 2, 3):
            _XTILES.append((_q, 2 * _P + _dj))
# DR hh-tile issue order: kp OUTER -- all kp0 tiles (consuming chain0's
# h chunks 0,1) run first, all kp1 tiles (chain1's chunks 2,3) last, so
# each chain's latency overlaps the other half's matmul stream. The
# steady-state cycle is then kp1-span (~1.5us) + chain latency instead
# of the full step.
_DRTILES = []
for _kp in range(KP):
    for _P in (0, 1):
        for _qs in ((0, 1), (2, 3)):       # IF group then GO (g before o)
            for _q in _qs:
                for _dj in (0, 1):
                    _DRTILES.append((_q, 2 * _P + _dj, _kp))


def _psum_slot(q, dj):
    if q in (0, 1):
        return "if", (q * 2 + dj) * BS
    return "go", ((q - 2) * 2 + dj) * BS


def pack_host(messages, W_ih, W_hh, b_ih, b_hh, W_fc, b_fc):
    # DR weights: whh8[p, tile, kh, m] = W_hh[rows(q,j)[m], kp*256+kh*128+p]
    whh8 = np.zeros((128, len(_DRTILES), 2, 128), np.float32)
    for idx, (q, j, kp) in enumerate(_DRTILES):
        rows = slice(q * H + j * 128, q * H + (j + 1) * 128)
        for kh in range(2):
            kk = kp * 256 + kh * 128
            whh8[:, idx, kh, :] = W_hh[rows, kk:kk + 128].T
    whh8 = np.clip(whh8 * WS, -240, 240).astype(ml_dtypes.float8_e4m3)
    whh8 = whh8.reshape(128, len(_DRTILES) * 2 * 128)

    bias = b_ih + b_hh
    wih = np.zeros((128, len(_XTILES) * 128), np.float32)
    for xm, (q, j) in enumerate(_XTILES):
        rows = slice(q * H + j * 128, q * H + (j + 1) * 128)
        r = xm % XNR
        wih[r * 32:r * 32 + V, xm * 128:(xm + 1) * 128] = W_ih[rows].T * WS
        wih[r * 32 + V, xm * 128:(xm + 1) * 128] = bias[rows] * WS
    wih = wih.astype(ml_dtypes.bfloat16)

    wfc = np.zeros((128, KT * NCLS), np.float32)
    for k in range(KT):
        wfc[:, k * NCLS:(k + 1) * NCLS] = W_fc.T[k * 128:(k + 1) * 128]
    wfc = wfc.astype(ml_dtypes.bfloat16)
    bfc = b_fc.astype(np.float32).reshape(NCLS, 1)

    in_maps = []
    for c in range(CORES):
        shard = messages[c * BS:(c + 1) * BS]                # [BS, T, V]
        xc = np.empty((KV, T, BS), np.float32)
        xc[:V] = shard.transpose(2, 1, 0)
        xc[V] = 1.0
        xc = xc.reshape(KV, T * BS).astype(ml_dtypes.bfloat16)
        in_maps.append({"x_c": xc, "whh8": whh8, "wih": wih,
                        "wfc": wfc, "bfc": bfc})
    return in_maps


def build(reps=1, nsteps=T):
    nc = bacc.Bacc("TRN2", target_bir_lowering=False, debug=False)

    x_dram = nc.dram_tensor("x_c", [KV, T * BS], BF16,
                            kind="ExternalInput").ap()
    whh_dram = nc.dram_tensor("whh8", [128, len(_DRTILES), 2, 128], FP8,
                              kind="ExternalInput").ap()
    wih_dram = nc.dram_tensor("wih", [128, len(_XTILES) * 128], BF16,
                              kind="ExternalInput").ap()
    wfc_dram = nc.dram_tensor("wfc", [128, KT * NCLS], BF16,
                              kind="ExternalInput").ap()
    bfc_dram = nc.dram_tensor("bfc", [NCLS, 1], F32,
                              kind="ExternalInput").ap()
    out_dram = nc.dram_tensor("out", [NCLS, BS], F32,
                              kind="ExternalOutput").ap()

    with tile.TileContext(nc) as tc:
        with (
            tc.tile_pool(name="const", bufs=1) as cpool,
            tc.tile_pool(name="xbuf", bufs=1) as xpool,
            tc.tile_pool(name="state", bufs=1) as spool,
            tc.tile_pool(name="psum", bufs=1, space="PSUM") as ppool,
            tc.tile_pool(name="work", bufs=1) as wpool,
        ):
            whh_sb = cpool.tile([128, len(_DRTILES), 2, 128], FP8)
            wih_sb = cpool.tile([128, len(_XTILES) * 128], BF16)
            wfc_sb = cpool.tile([128, KT * NCLS], BF16)
            bfc_sb = cpool.tile([NCLS, 1], F32)
            x4_sb = xpool.tile([128, T * BS], BF16)

            # h buffers: fp8, addressed as [128, kpair, khalf, BS]
            h_sb = [spool.tile([128, KP, 2, BS], FP8, name=f"h{i}")
                    for i in range(2)]
            hfinb = spool.tile([128, KT * BS], BF16)
            c_sb = spool.tile([128, KT * BS], BF16)

            ntile4 = len(_DRTILES) // 4
            for i in range(4):
                ts_ = slice(i * ntile4, (i + 1) * ntile4)
                nc.sync.dma_start(whh_sb[:, ts_], whh_dram[:, ts_])
            nc.sync.dma_start(wih_sb[:], wih_dram[:])
            nc.sync.dma_start(wfc_sb[:], wfc_dram[:])
            nc.sync.dma_start(bfc_sb[:], bfc_dram[:])
            xc_cols = T * BS // 8
            for i in range(8):
                cols = slice(i * xc_cols, (i + 1) * xc_cols)
                for r in range(XNR):
                    nc.sync.dma_start(x4_sb[r * 32:r * 32 + KV, cols],
                                      x_dram[:, cols])

            pif = [ppool.tile([128, 4 * BS], F32, name=f"pif{p}")
                   for p in range(2)]
            pgo = [ppool.tile([128, 4 * BS], F32, name=f"pgo{p}")
                   for p in range(2)]

            sif = [wpool.tile([128, 4 * BS], BF16, name=f"sif{p}")
                   for p in range(2)]
            g_t = [wpool.tile([128, 2 * BS], BF16, name=f"g{p}")
                   for p in range(2)]
            o_t = [wpool.tile([128, 2 * BS], BF16, name=f"o{p}")
                   for p in range(2)]
            ig_t = [wpool.tile([128, 2 * BS], BF16, name=f"ig{p}")
                    for p in range(2)]
            fc_t = [wpool.tile([128, 2 * BS], BF16, name=f"fc{p}")
                    for p in range(2)]
            tc_t = [wpool.tile([128, 2 * BS], BF16, name=f"tc{p}")
                    for p in range(2)]

            def x_mm(t, xm_base):
                xs = slice(t * BS, (t + 1) * BS)
                for n in range(4):
                    xm = xm_base + n
                    q, j = _XTILES[xm]
                    kind, col = _psum_slot(q, j % 2)
                    dst = (pif if kind == "if" else pgo)[j // 2]
                    r = xm % XNR
                    nc.tensor.matmul(
                        dst[:, col:col + BS],
                        wih_sb[r * 32:r * 32 + KV,
                               xm * 128:(xm + 1) * 128],
                        x4_sb[r * 32:r * 32 + KV, xs],
                        start=(col % 512 == 0), stop=False,
                        tile_position=(r * 32, 0),
                    )

            def hh_mm(hr, idx_base, n_mms=8):
                """Issue DoubleRow hh matmuls idx_base..+n_mms."""
                for n in range(n_mms):
                    idx = idx_base + n
                    q, j, kp = _DRTILES[idx]
                    kind, col = _psum_slot(q, j % 2)
                    dst = (pif if kind == "if" else pgo)[j // 2]
                    last = (kp == KP - 1) and (col % 512 == BS)
                    nc.tensor.matmul(
                        dst[:, col:col + BS],
                        whh_sb[:, idx],
                        hr[:, kp],
                        start=False, stop=last,
                        perf_mode=DR,
                    )

            def chain(P, hw, write_fin):
                s2 = slice(P * 2 * BS, (P + 1) * 2 * BS)
                nc.scalar.activation(sif[P][:], pif[P][:], AF.Sigmoid,
                                     scale=1.0 / WS)
                nc.scalar.activation(g_t[P][:], pgo[P][:, 0:2 * BS],
                                     AF.Tanh, scale=1.0 / WS)
                # fc on DVE, issued before ig: it only needs sif, so it
                # runs while ACT computes tanh(g); ~0.3us vs 1.1 gpsimd.
                nc.vector.tensor_mul(fc_t[P][:], sif[P][:, 2 * BS:4 * BS],
                                     c_sb[:, s2])
                nc.vector.tensor_mul(ig_t[P][:], sif[P][:, 0:2 * BS],
                                     g_t[P][:])
                nc.scalar.activation(o_t[P][:], pgo[P][:, 2 * BS:4 * BS],
                                     AF.Sigmoid, scale=1.0 / WS)
                nc.vector.tensor_add(c_sb[:, s2], ig_t[P][:], fc_t[P][:])
                nc.scalar.activation(tc_t[P][:], c_sb[:, s2], AF.Tanh)
                nc.vector.tensor_mul(hw[:, P], o_t[P][:], tc_t[P][:])
                if write_fin:
                    nc.vector.tensor_mul(hfinb[:, s2], o_t[P][:],
                                         tc_t[P][:])

            for rep in range(reps):
                nc.vector.memset(h_sb[0][:], 0.0)
                nc.vector.memset(c_sb[:], 0.0)
                for t in range(nsteps):
                    hr, hw = h_sb[t % 2], h_sb[1 - t % 2]
                    fin = (t == nsteps - 1)
                    x_mm(t, 0)       # x -> pif0+pgo0, dj=0
                    x_mm(t, 4)       # x -> pif0+pgo0, dj=1
                    x_mm(t, 8)       # x -> pif1+pgo1, dj=0
                    x_mm(t, 12)      # x -> pif1+pgo1, dj=1
                    hh_mm(hr, 0, 16)   # kp0: all four psum groups
                    hh_mm(hr, 16, 16)  # kp1: all four psum groups
                    chain(0, hw, fin)
                    chain(1, hw, fin)

            for k in range(KT):
                nc.tensor.matmul(
                    pif[0][0:NCLS, 0:BS],
                    wfc_sb[:, k * NCLS:(k + 1) * NCLS],
                    hfinb[:, k * BS:(k + 1) * BS],
                    start=(k == 0), stop=(k == KT - 1),
                )
            out_sb = cpool.tile([NCLS, BS], F32)
            nc.scalar.activation(out_sb[:], pif[0][0:NCLS, 0:BS],
                                 AF.Identity, bias=bfc_sb[:])
            nc.sync.dma_start(out_dram[:], out_sb[:])

    nc.compile()
    return nc


_NC_CACHE = None


def kernel(messages, W_ih, W_hh, b_ih, b_hh, W_fc, b_fc):
    """Full-input entry point: shard, run on 8 NeuronCores, gather."""
    global _NC_CACHE
    messages = np.asarray(messages, np.float32)
    W_ih = np.asarray(W_ih, np.float32)
    W_hh = np.asarray(W_hh, np.float32)
    b_ih = np.asarray(b_ih, np.float32)
    b_hh = np.asarray(b_hh, np.float32)
    W_fc = np.asarray(W_fc, np.float32)
    b_fc = np.asarray(b_fc, np.float32)

    in_maps = pack_host(messages, W_ih, W_hh, b_ih, b_hh, W_fc, b_fc)
    if _NC_CACHE is None:
        _NC_CACHE = build(1)
    res = run_bass_kernel_spmd(_NC_CACHE, in_maps, list(range(CORES)))
    outs = [np.ascontiguousarray(np.asarray(res.results[c]["out"]).T)
            for c in range(CORES)]
    return np.concatenate(outs, axis=0).astype(np.float32)
